# revision 19
# baseline (speedup 1.0000x reference)
"""Trainium2 Bass kernel for nn_Attention_66108136620268.

Full multi-head attention (B=32, N=512, C=768, H=12) with CLS-attention
top-k token pruning, data-parallel over batch across 8 NeuronCores.

Main attention/proj path runs in bf16 (out tolerance is loose); the
CLS-score path that determines the top-k ordering is computed in exact
fp32 via a fused form:  l[h,k] = sum_a V6[a,h] * xT[a,k]  with
V6 = Wk @ blockdiag(q0),  q0 = Wq^T @ x[:,0,:].  q0/V6 depend only on
the CLS rows of x, so they are batched over all samples at kernel init.

Top-k is computed as an exact rank sort: rank[i] = #{s_j > s_i} +
#{j<i: s_j == s_i} via fused compare+accumulate DVE ops, then inverted
with an equality-match matmul; positions >= ceil(0.7*valid) padded 1e9.

The per-sample pipeline is software-pipelined at emission level: the
attention/normalize phase of sample b-1 is interleaved with the
projection phases of sample b so the TensorEngine sees dense work.

kernel(**inputs) takes the FULL inputs, shards over 8 cores, returns
(out, index, idx, BOUNDARY) matching the reference.
"""

import numpy as np

B, N, C, H = 32, 512, 768, 12
HD = C // H                      # 64
SCALE = float(HD ** -0.5)        # 0.125
KEEP_RATE = 0.7
BOUNDARY = 358
NCORES = 8
BL = B // NCORES                 # 4 samples per core
NC1 = N - 1                      # 511

_CACHE = {}


def _build_nc():
    import concourse.bacc as bacc
    import concourse.bass as bass
    import concourse.mybir as mybir
    import concourse.tile as tile
    from concourse.masks import make_identity

    F32 = mybir.dt.float32
    BF16 = mybir.dt.bfloat16
    FP16 = mybir.dt.float16
    I32 = mybir.dt.int32
    AL = mybir.AluOpType
    AF = mybir.ActivationFunctionType
    PSUM = bass.MemorySpace.PSUM

    nc = bacc.Bacc()

    x_d = nc.declare_dram_parameter("x", [BL, N, C], F32, isOutput=False)
    mrow_d = nc.declare_dram_parameter("mask_row", [BL, N], I32, isOutput=False)
    wqkv_d = nc.declare_dram_parameter("w_qkv", [C, 3 * C], F32, isOutput=False)
    wproj_d = nc.declare_dram_parameter("w_proj", [C, C], F32, isOutput=False)
    bproj_d = nc.declare_dram_parameter("b_proj", [1, C], F32, isOutput=False)
    out_d = nc.declare_dram_parameter("out_x", [BL, N, C], F32, isOutput=True)
    idx_d = nc.declare_dram_parameter("out_idx", [BL, BOUNDARY], F32, isOutput=True)

    NCH = C // 128               # 6 c-chunks
    NTOK = N // 128              # 4 token chunks

    with tile.TileContext(nc) as tc:
        with (
            tc.tile_pool(name="const", bufs=1) as cp,
            tc.tile_pool(name="wp", bufs=1) as wp,
            tc.tile_pool(name="st", bufs=1) as stp,
            tc.tile_pool(name="ph", bufs=1) as php,
            tc.tile_pool(name="sm", bufs=1) as smp,
            tc.tile_pool(name="po", bufs=2) as pop,
            tc.tile_pool(name="ps_s", bufs=1, space=PSUM) as ps_s,
            tc.tile_pool(name="ps_o", bufs=2, space=PSUM) as ps_o,
            tc.tile_pool(name="ps_b", bufs=2, space=PSUM) as ps_b,
        ):
            # ---------------- constants ----------------
            identf = cp.tile([128, 128], F32, tag="identf")
            make_identity(nc, identf[:])
            ones_col = cp.tile([1, 128], F32, tag="ones_col")
            nc.vector.memset(ones_col[:], 1.0)
            ones_col_bf = cp.tile([1, 128], BF16, tag="ones_col_bf")
            nc.vector.memset(ones_col_bf[:], 1.0)
            ones12 = cp.tile([1, 12], F32, tag="ones12")
            nc.vector.memset(ones12[:], 1.0)
            iota_i = cp.tile([128, 512], I32, tag="iota_i")
            nc.gpsimd.iota(iota_i[:], pattern=[[1, 512]], base=0, channel_multiplier=0)
            iota_row = cp.tile([128, 512], F32, tag="iota_row")
            nc.vector.tensor_copy(iota_row[:], iota_i[:])
            pos_i = cp.tile([128, 4], I32, tag="pos_i")
            nc.gpsimd.iota(pos_i[:], pattern=[[128, 4]], base=0, channel_multiplier=1)
            pos_f = cp.tile([128, 4], F32, tag="pos_f")
            nc.vector.tensor_copy(pos_f[:], pos_i[:])
            pos_h = cp.tile([128, 4], FP16, tag="pos_h")
            nc.vector.tensor_copy(pos_h[:], pos_i[:])
            pad1e9 = cp.tile([1, 512], F32, tag="pad1e9")
            nc.vector.memset(pad1e9[:], 1.0e9)
            jlts = []
            for c in range(4):
                jl = cp.tile([128, 512], F32, tag=f"jlt{c}", name=f"jlt{c}")
                nc.vector.memset(jl[:], 1.0)
                nc.gpsimd.affine_select(
                    out=jl[:], in_=jl[:], compare_op=AL.is_gt,
                    fill=0.0, base=c * 128, pattern=[[-1, 512]],
                    channel_multiplier=1)
                jlts.append(jl)

            # ---------------- init: weights + batched q0 / V6 ----------
            xcls = smp.tile([4, C], F32, tag="xcls")
            for b in range(BL):
                nc.sync.dma_start(xcls[b:b + 1, :], x_d[b, 0:1, :])
            xclsT = smp.tile([128, 24], F32, tag="xclsT")
            xct_ps = ps_b.tile([128, 512], F32, tag="bigps")
            for j in range(NCH):
                nc.tensor.transpose(xct_ps[:, j * 4:(j + 1) * 4],
                                    xcls[0:4, j * 128:(j + 1) * 128],
                                    identf[0:4, 0:4])
            nc.scalar.copy(xclsT[:], xct_ps[:, 0:24])

            wqkv = []
            wproj = []
            q0_both = ps_s.tile([4, 2, 512], F32, tag="s2", bufs=2,
                                name="q0_both")
            q0_ps = q0_both[:, 0, :]
            q0_ps2 = q0_both[:, 1, 0:256]
            for j in range(NCH):
                wst = stp.tile([128, 3 * C], F32, tag="xstage0", name=f"wst{j}")
                nc.sync.dma_start(wst[:], wqkv_d[j * 128:(j + 1) * 128, :])
                w = wp.tile([128, 3 * C], BF16, tag=f"wqkv{j}", name=f"wqkv{j}")
                nc.vector.tensor_copy(w[:], wst[:])
                wqkv.append(w)
                nc.tensor.matmul(q0_ps, xclsT[:, j * 4:(j + 1) * 4],
                                 wst[:, 0:512],
                                 start=(j == 0), stop=(j == NCH - 1))
                nc.tensor.matmul(q0_ps2, xclsT[:, j * 4:(j + 1) * 4],
                                 wst[:, 512:C],
                                 start=(j == 0), stop=(j == NCH - 1))
            q0row = smp.tile([4, C], F32, tag="q0row")
            nc.scalar.copy(q0row[:, 0:512], q0_ps)
            nc.scalar.copy(q0row[:, 512:C], q0_ps2)
            for j in range(NCH):
                wst2 = stp.tile([128, C], F32, tag="xstage1", name=f"wst2{j}")
                nc.sync.dma_start(wst2[:], wproj_d[j * 128:(j + 1) * 128, :])
                w = wp.tile([128, C], BF16, tag=f"wproj{j}", name=f"wproj{j}")
                nc.vector.tensor_copy(w[:], wst2[:])
                wproj.append(w)
            # WkT (fp32, init-only): shares the xT tags so the space is
            # reclaimed once the per-sample xT tiles allocate.
            wkT = [php.tile([128, C], F32, tag=f"xT{bb}", name=f"wkT{bb}")
                   for bb in range(NCH)]
            for j in range(NCH):
                wst3 = stp.tile([128, C], F32, tag="xstage2", name=f"wst3{j}")
                nc.sync.dma_start(wst3[:], wqkv_d[j * 128:(j + 1) * 128, C:2 * C])
                wt_ps = ps_b.tile([128, 512], F32, tag="bigps")
                for bb in range(NCH):
                    sl = bb % 4
                    if bb == 4:
                        wt_ps = ps_b.tile([128, 512], F32, tag="bigps")
                    nc.tensor.transpose(
                        wt_ps[:, sl * 128:(sl + 1) * 128],
                        wst3[:, bb * 128:(bb + 1) * 128], identf[:])
                    nc.scalar.copy(wkT[bb][:, j * 128:(j + 1) * 128],
                                   wt_ps[:, sl * 128:(sl + 1) * 128])
            q0col = smp.tile([128, 24], F32, tag="q0col")
            q0c_ps = ps_b.tile([128, 512], F32, tag="bigps")
            for j in range(NCH):
                nc.tensor.transpose(q0c_ps[:, j * 4:(j + 1) * 4],
                                    q0row[0:4, j * 128:(j + 1) * 128],
                                    identf[0:4, 0:4])
            nc.scalar.copy(q0col[:], q0c_ps[:, 0:24])
            q0b_all = smp.tile([128, 6 * 48], F32, tag="q0b_all")
            nc.vector.memset(q0b_all[:], 0.0)
            for b in range(BL):
                for h in range(12):
                    j = h // 2
                    p0 = (h % 2) * 64
                    nc.vector.tensor_copy(
                        q0b_all[p0:p0 + 64, j * 48 + b * 12 + h:
                                j * 48 + b * 12 + h + 1],
                        q0col[p0:p0 + 64, j * 4 + b:j * 4 + b + 1])
            v6_all = []
            for a in range(NCH):
                v6_ps = ps_b.tile([128, 512], F32, tag="bigps")
                for bb in range(NCH):
                    nc.tensor.matmul(
                        v6_ps[:, 0:48],
                        wkT[bb][:, a * 128:(a + 1) * 128],
                        q0b_all[:, bb * 48:(bb + 1) * 48],
                        start=(bb == 0), stop=(bb == NCH - 1))
                v6a = wp.tile([128, 48], F32, tag=f"v6a{a}", name=f"v6a{a}")
                nc.scalar.copy(v6a[:], v6_ps[:, 0:48])
                v6_all.append(v6a)
            bias_row = smp.tile([1, C], F32, tag="bias_row")
            nc.sync.dma_start(bias_row[:], bproj_d[:])
            bias_bc = wp.tile([128, C], BF16, tag="bias_bc")
            for n0, n1 in ((0, 512), (512, 768)):
                bb_ps = ps_b.tile([128, 512], F32, tag="bigps")
                nc.tensor.matmul(bb_ps[:, :n1 - n0], ones_col[:],
                                 bias_row[:, n0:n1], start=True, stop=True)
                nc.scalar.copy(bias_bc[:, n0:n1], bb_ps[:, :n1 - n0])

            # ---------------- per-sample state ----------------
            state = {}

            def units_front(b):
                """Phases A-D + F for sample b, as emission units."""
                st = state[b] = {}
                units = []

                def u_load():
                    stg = [stp.tile([128, C], F32, tag=f"xstage{t}",
                                    name=f"stg{b}_{t}") for t in range(NTOK)]
                    for t in range(NTOK):
                        nc.sync.dma_start(stg[t][:],
                                          x_d[b, t * 128:(t + 1) * 128, :])
                    st["stg"] = stg
                    mask_i = smp.tile([1, 512], I32, tag="mask_i",
                                      name=f"mask_i{b}")
                    nc.sync.dma_start(mask_i[:], mrow_d[b:b + 1, :])
                    mask_f = smp.tile([1, 512], F32, tag="mask_f", bufs=2,
                                      name=f"mask_f{b}")
                    nc.vector.tensor_copy(mask_f[:], mask_i[:])
                    mcol_i = smp.tile([128, 4], I32, tag="mcol_i",
                                      name=f"mcol_i{b}")
                    nc.sync.dma_start(
                        mcol_i[:], mrow_d[b].rearrange("(c p) -> p c", p=128))
                    mcol = smp.tile([128, 4], F32, tag="mcol", bufs=2,
                                    name=f"mcol{b}")
                    nc.vector.tensor_copy(mcol[:], mcol_i[:])
                    st["mask_f"] = mask_f
                    st["mcol"] = mcol
                    st["xT"] = [php.tile([128, 512], F32, tag=f"xT{j}",
                                         name=f"xT{b}_{j}") for j in range(NCH)]
                    st["xTb"] = [php.tile([128, 512], BF16, tag=f"xTb{j}",
                                          name=f"xTb{b}_{j}") for j in range(NCH)]
                    st["qkT"] = [php.tile([128, 512], BF16, tag=f"qkT{m}",
                                          bufs=2, name=f"qkT{b}_{m}")
                                 for m in range(12)]
                    st["v_sb"] = [php.tile([128, 12 * 65], BF16, tag=f"v{t}",
                                           bufs=2, name=f"v{b}_{t}")
                                  for t in range(NTOK)]
                units.append(u_load)

                def u_transp(j):
                    def f():
                        xt_ps = ps_b.tile([128, 512], F32, tag="bigps")
                        for t in range(NTOK):
                            nc.tensor.transpose(
                                xt_ps[:, t * 128:(t + 1) * 128],
                                st["stg"][t][:, j * 128:(j + 1) * 128],
                                identf[:])
                        nc.scalar.copy(st["xT"][j][:], xt_ps[:])
                        nc.scalar.copy(st["xTb"][j][:], xt_ps[:])
                    return f
                units.extend(u_transp(j) for j in range(NCH))

                def u_qk(m):
                    def f():
                        qk_ps = ps_b.tile([128, 512], F32, tag="bigps")
                        for j in range(NCH):
                            nc.tensor.matmul(
                                qk_ps[:], wqkv[j][:, m * 128:(m + 1) * 128],
                                st["xTb"][j][:],
                                start=(j == 0), stop=(j == NCH - 1))
                        nc.scalar.copy(st["qkT"][m][:], qk_ps[:])
                    return f
                units.extend(u_qk(m) for m in range(12))

                def u_v(t):
                    def f():
                        for n in range(2):
                            v_ps = ps_b.tile([128, 512], F32, tag="bigps")
                            nf0 = 2 * C + n * 384
                            for j in range(NCH):
                                nc.tensor.matmul(
                                    v_ps[:, :384],
                                    st["xTb"][j][:, t * 128:(t + 1) * 128],
                                    wqkv[j][:, nf0:nf0 + 384],
                                    start=(j == 0), stop=(j == NCH - 1))
                            dst = st["v_sb"][t][:, n * 390:(n + 1) * 390]
                            dst = dst.rearrange("p (h d) -> p h d", h=6)[:, :, 0:64]
                            nc.vector.tensor_scalar(
                                dst,
                                v_ps[:, :384].rearrange("p (h d) -> p h d", h=6),
                                st["mcol"][:, t:t + 1], None,
                                op0=AL.mult, op1=AL.bypass)
                        mdst = st["v_sb"][t].rearrange(
                            "p (h d) -> p h d", h=12)[:, :, 64:65]
                        nc.vector.tensor_copy(
                            mdst, st["mcol"][:, t:t + 1].to_broadcast([128, 12, 1]))
                    return f
                units.extend(u_v(t) for t in range(NTOK))

                def u_scores():
                    scls_ps = ps_b.tile([128, 512], F32, tag="bigps")
                    for a in range(NCH):
                        nc.tensor.matmul(
                            scls_ps[0:12, :],
                            v6_all[a][:, b * 12:(b + 1) * 12],
                            st["xT"][a][:], start=(a == 0), stop=(a == NCH - 1))
                    mask_bc_ps = ps_b.tile([128, 512], F32, tag="bigps")
                    nc.tensor.matmul(mask_bc_ps[0:12, :], ones12[:],
                                     st["mask_f"][:], start=True, stop=True)
                    pexp = smp.tile([12, 512], F32, tag="pexp")
                    nc.scalar.activation(pexp[:], scls_ps[0:12, :], AF.Exp,
                                         scale=SCALE)
                    pm = smp.tile([12, 512], F32, tag="pm")
                    den12 = smp.tile([12, 1], F32, tag="den12")
                    nc.vector.scalar_tensor_tensor(
                        out=pm[:], in0=pexp[:], scalar=1.0,
                        in1=mask_bc_ps[0:12, :],
                        op0=AL.mult, op1=AL.mult, accum_out=den12[:])
                    rw = smp.tile([12, 1], F32, tag="rw")
                    nc.vector.reciprocal(rw[:], den12[:])
                    nc.vector.tensor_scalar_mul(rw[:], rw[:], 1.0 / 12.0)
                    sc_ps = ps_b.tile([128, 512], F32, tag="bigps")
                    nc.tensor.matmul(sc_ps[0:1, 0:NC1], rw[:], pm[:, 1:512],
                                     start=True, stop=True)
                    s_ext = smp.tile([1, 512], F32, tag="s_ext")
                    nc.scalar.copy(s_ext[0:1, 0:NC1], sc_ps[0:1, 0:NC1])
                    nc.vector.memset(s_ext[0:1, NC1:512], -1.0)
                    st["s_ext"] = s_ext
                units.append(u_scores)

                def u_rank1():
                    sb_ps = ps_b.tile([128, 512], F32, tag="bigps")
                    nc.tensor.matmul(sb_ps[:], ones_col[:], st["s_ext"][:],
                                     start=True, stop=True)
                    s_bc = smp.tile([128, 512], F32, tag="s_bc")
                    nc.scalar.copy(s_bc[:], sb_ps[:])
                    junk = smp.tile([128, 512], F32, tag="junk")
                    s_col = smp.tile([128, 4], F32, tag="s_col")
                    for c in range(4):
                        nc.vector.scalar_tensor_tensor(
                            out=junk[:], in0=iota_row[:],
                            scalar=pos_f[:, c:c + 1],
                            in1=s_bc[:], op0=AL.is_equal, op1=AL.mult,
                            accum_out=s_col[:, c:c + 1])
                    st["s_bc"] = s_bc
                    st["s_col"] = s_col
                    st["junk"] = junk
                units.append(u_rank1)

                def u_rank2():
                    s_bc, s_col, junk = st["s_bc"], st["s_col"], st["junk"]
                    # eq-stt shares the junk scratch; DVE ops serialize anyway
                    rank_col = smp.tile([128, 4], F32, tag="rank_col")
                    rank_eq = smp.tile([128, 4], F32, tag="rank_eq")
                    for c in range(4):
                        nc.vector.tensor_scalar(
                            junk[:], s_bc[:], s_col[:, c:c + 1], None,
                            op0=AL.is_gt, op1=AL.add,
                            accum_out=rank_col[:, c:c + 1])
                        nc.vector.scalar_tensor_tensor(
                            out=junk[:], in0=s_bc[:], scalar=s_col[:, c:c + 1],
                            in1=jlts[c][:], op0=AL.is_equal, op1=AL.mult,
                            accum_out=rank_eq[:, c:c + 1])
                    nc.vector.tensor_add(rank_col[:], rank_col[:], rank_eq[:])
                    idx_ps = ps_b.tile([128, 512], F32, tag="bigps")
                    for c in range(4):
                        eT = smp.tile([128, 512], FP16, tag="eT",
                                      name=f"eT{b}_{c}")
                        nc.vector.tensor_scalar(
                            eT[:], iota_row[:], rank_col[:, c:c + 1], None,
                            op0=AL.is_equal, op1=AL.bypass)
                        nc.tensor.matmul(idx_ps[0:1, :], pos_h[:, c:c + 1],
                                         eT[:], start=(c == 0), stop=(c == 3))
                    cnt = smp.tile([1, 1], F32, tag="cnt")
                    nc.vector.reduce_sum(cnt[:], st["mask_f"][0:1, 1:512],
                                         axis=mybir.AxisListType.X)
                    y = smp.tile([1, 1], F32, tag="y")
                    nc.vector.tensor_scalar_mul(y[:], cnt[:],
                                                float(np.float32(KEEP_RATE)))
                    sel = smp.tile([1, 512], I32, tag="sel")
                    nc.vector.tensor_scalar(
                        sel[:], iota_row[0:1, :], y[:, 0:1], None,
                        op0=AL.is_lt, op1=AL.bypass)
                    idx_sb = smp.tile([1, 512], F32, tag="idx_sb")
                    nc.scalar.copy(idx_sb[:], idx_ps[0:1, :])
                    idx_fin = smp.tile([1, 512], F32, tag="idx_fin")
                    nc.vector.select(idx_fin[:], sel[:], idx_sb[:], pad1e9[:])
                    nc.sync.dma_start(idx_d[b:b + 1, :],
                                      idx_fin[0:1, 0:BOUNDARY])
                units.append(u_rank2)
                return units

            def units_back(b):
                """Phases E (attention) + G (proj) for sample b."""
                st = state[b]
                units = []

                def mk_oallT():
                    st["oallT"] = [php.tile([128, 512], BF16, tag=f"oT{j}",
                                            bufs=2, name=f"oT{b}_{j}")
                                   for j in range(NCH)]
                units.append(mk_oallT)

                def u_pair(hp):
                    def f():
                        qkT = st["qkT"]
                        v_sb = st["v_sb"]
                        j = hp
                        p_chunks = []
                        for c in range(4):
                            s_ps = ps_s.tile([128, 2, 512], F32, tag="s2",
                                             bufs=2, name=f"s2_{b}_{hp}_{c}")
                            nc.tensor.matmul(
                                s_ps[:, 0, :],
                                qkT[6 + j][0:64, c * 128:(c + 1) * 128],
                                qkT[j][0:64, :], start=True, stop=True)
                            nc.tensor.matmul(
                                s_ps[:, 1, :],
                                qkT[6 + j][64:128, c * 128:(c + 1) * 128],
                                qkT[j][64:128, :], start=True, stop=True)
                            p_sb = stp.tile([128, 2, 512], BF16, tag="p_sb",
                                            bufs=4, name=f"p_{b}_{hp}_{c}")
                            nc.scalar.activation(p_sb[:], s_ps[:], AF.Exp,
                                                 scale=SCALE)
                            p_chunks.append(p_sb)
                        for hh in range(2):
                            h = 2 * hp + hh
                            p0 = hh * 64
                            o_ps = ps_o.tile([65, 512], F32, tag="ops")
                            for c in range(4):
                                nc.tensor.matmul(
                                    o_ps[:], v_sb[c][:, h * 65:h * 65 + 65],
                                    p_chunks[c][:, hh, :],
                                    start=(c == 0), stop=(c == 3))
                            den_row = stp.tile([1, 512], F32, tag="den_row",
                                               bufs=2)
                            nc.vector.tensor_copy(den_row[:], o_ps[64:65, :])
                            rec_row = stp.tile([1, 512], F32, tag="rec_row",
                                               bufs=2)
                            nc.vector.reciprocal_approx_fast(out=rec_row[:],
                                                             in_=den_row[:])
                            rec_bf = stp.tile([1, 512], BF16, tag="rec_bf",
                                              bufs=2)
                            nc.vector.tensor_copy(rec_bf[:], rec_row[:])
                            rb_ps = ps_o.tile([65, 512], F32, tag="ops",
                                              name=f"rb_{b}_{h}")
                            nc.tensor.matmul(rb_ps[0:64, :],
                                             ones_col_bf[:, 0:64],
                                             rec_bf[:], start=True, stop=True)
                            rb_sb = stp.tile([64, 512], F32, tag="rb_sb",
                                             bufs=2)
                            nc.scalar.copy(rb_sb[:], rb_ps[0:64, :])
                            nc.vector.tensor_tensor(
                                out=st["oallT"][j][p0:p0 + 64, :],
                                in0=o_ps[0:64, :], in1=rb_sb[:], op=AL.mult)
                    return f
                units.extend(u_pair(hp) for hp in range(6))

                def u_proj(t):
                    def f():
                        pr_sb = pop.tile([128, C], F32, tag="proj")
                        for n in range(2):
                            pr_ps = ps_b.tile([128, 512], F32, tag="bigps")
                            n0 = n * 384
                            for j in range(NCH):
                                nc.tensor.matmul(
                                    pr_ps[:, :384],
                                    st["oallT"][j][:, t * 128:(t + 1) * 128],
                                    wproj[j][:, n0:n0 + 384],
                                    start=(j == 0), stop=(j == NCH - 1))
                            nc.vector.tensor_tensor(
                                out=pr_sb[:, n0:n0 + 384], in0=pr_ps[:, :384],
                                in1=bias_bc[:, n0:n0 + 384], op=AL.add)
                        nc.sync.dma_start(
                            out_d[b, t * 128:(t + 1) * 128, :], pr_sb[:])
                    return f
                units.extend(u_proj(t) for t in range(NTOK))
                return units

            # software-pipelined emission: back(b-1) interleaved with front(b)
            prev_back = None
            for b in range(BL):
                front = units_front(b)
                if prev_back is None:
                    for u in front:
                        u()
                else:
                    nf, nb = len(front), len(prev_back)
                    fi = bi = 0
                    while fi < nf or bi < nb:
                        # pace front units against back units
                        if bi < nb and (fi * nb >= bi * nf or fi >= nf):
                            prev_back[bi]()
                            bi += 1
                        else:
                            front[fi]()
                            fi += 1
                prev_back = units_back(b)
            for u in prev_back:
                u()

    nc.compile()
    return nc


def _get_nc():
    if "nc" not in _CACHE:
        _CACHE["nc"] = _build_nc()
    return _CACHE["nc"]


def _make_in_maps(inputs):
    x = np.ascontiguousarray(np.asarray(inputs["x"], dtype=np.float32))
    mask_row = np.ascontiguousarray(
        np.asarray(inputs["attn_mask"], dtype=np.int32)[:, 0, :])
    w_qkv = np.ascontiguousarray(np.asarray(inputs["w_qkv"], dtype=np.float32))
    w_proj = np.ascontiguousarray(np.asarray(inputs["w_proj"], dtype=np.float32))
    b_proj = np.ascontiguousarray(
        np.asarray(inputs["b_proj"], dtype=np.float32).reshape(1, C))
    in_maps = []
    for i in range(NCORES):
        sl = slice(i * BL, (i + 1) * BL)
        in_maps.append({
            "x": x[sl],
            "mask_row": mask_row[sl],
            "w_qkv": w_qkv,
            "w_proj": w_proj,
            "b_proj": b_proj,
        })
    return in_maps


def run_on_device(inputs, trace=False, tmpdir=None):
    """Build + run on the 8 NeuronCores; returns (out, idx, exec_time_ns)."""
    from concourse.bass_utils import run_bass_kernel_spmd

    nc = _get_nc()
    in_maps = _make_in_maps(inputs)
    res = run_bass_kernel_spmd(
        nc, in_maps, core_ids=list(range(NCORES)), trace=trace, tmpdir=tmpdir)
    out = np.concatenate([res.results[i]["out_x"] for i in range(NCORES)], axis=0)
    idx = np.concatenate([res.results[i]["out_idx"] for i in range(NCORES)], axis=0)
    return out, idx, res.exec_time_ns


def kernel(**inputs):
    out, idx, _ = run_on_device(inputs, trace=False)
    index = np.ascontiguousarray(
        np.broadcast_to(idx[:, :, None], (B, BOUNDARY, C)))
    return out, index, idx, BOUNDARY


# revision 20
# speedup vs baseline: 1.0378x; 1.0378x over previous
"""Trainium2 Bass kernel for nn_Attention_66108136620268.

Full multi-head attention (B=32, N=512, C=768, H=12) with CLS-attention
top-k token pruning, data-parallel over batch across 8 NeuronCores.

Main attention/proj path runs in bf16 (out tolerance is loose); the
CLS-score path that determines the top-k ordering is computed in exact
fp32 via a fused form:  l[h,k] = sum_a V6[a,h] * xT[a,k]  with
V6 = Wk @ blockdiag(q0),  q0 = Wq^T @ x[:,0,:].  q0/V6 depend only on
the CLS rows of x, so they are batched over all samples at kernel init.

Top-k is computed as an exact rank sort: rank[i] = #{s_j > s_i} +
#{j<i: s_j == s_i} via fused compare+accumulate DVE ops, then inverted
with an equality-match matmul; positions >= ceil(0.7*valid) padded 1e9.

The per-sample pipeline is software-pipelined at emission level: the
attention/normalize phase of sample b-1 is interleaved with the
projection phases of sample b so the TensorEngine sees dense work.

kernel(**inputs) takes the FULL inputs, shards over 8 cores, returns
(out, index, idx, BOUNDARY) matching the reference.
"""

import numpy as np

B, N, C, H = 32, 512, 768, 12
HD = C // H                      # 64
SCALE = float(HD ** -0.5)        # 0.125
KEEP_RATE = 0.7
BOUNDARY = 358
NCORES = 8
BL = B // NCORES                 # 4 samples per core
NC1 = N - 1                      # 511

_CACHE = {}


def _build_nc():
    import concourse.bacc as bacc
    import concourse.bass as bass
    import concourse.mybir as mybir
    import concourse.tile as tile
    from concourse.masks import make_identity

    F32 = mybir.dt.float32
    BF16 = mybir.dt.bfloat16
    FP16 = mybir.dt.float16
    I32 = mybir.dt.int32
    AL = mybir.AluOpType
    AF = mybir.ActivationFunctionType
    PSUM = bass.MemorySpace.PSUM

    nc = bacc.Bacc()

    x_d = nc.declare_dram_parameter("x", [BL, N, C], F32, isOutput=False)
    mrow_d = nc.declare_dram_parameter("mask_row", [BL, N], I32, isOutput=False)
    wqkv_d = nc.declare_dram_parameter("w_qkv", [C, 3 * C], F32, isOutput=False)
    wproj_d = nc.declare_dram_parameter("w_proj", [C, C], F32, isOutput=False)
    bproj_d = nc.declare_dram_parameter("b_proj", [1, C], F32, isOutput=False)
    out_d = nc.declare_dram_parameter("out_x", [BL, N, C], F32, isOutput=True)
    idx_d = nc.declare_dram_parameter("out_idx", [BL, BOUNDARY], F32, isOutput=True)

    NCH = C // 128               # 6 c-chunks
    NTOK = N // 128              # 4 token chunks

    with tile.TileContext(nc) as tc:
        with (
            tc.tile_pool(name="const", bufs=1) as cp,
            tc.tile_pool(name="wp", bufs=1) as wp,
            tc.tile_pool(name="st", bufs=1) as stp,
            tc.tile_pool(name="ph", bufs=1) as php,
            tc.tile_pool(name="sm", bufs=1) as smp,
            tc.tile_pool(name="po", bufs=2) as pop,
            tc.tile_pool(name="ps_s", bufs=1, space=PSUM) as ps_s,
            tc.tile_pool(name="ps_o", bufs=2, space=PSUM) as ps_o,
            tc.tile_pool(name="ps_b", bufs=2, space=PSUM) as ps_b,
        ):
            # ---------------- constants ----------------
            identf = cp.tile([128, 128], F32, tag="identf")
            make_identity(nc, identf[:])
            ones_col = cp.tile([1, 128], F32, tag="ones_col")
            nc.vector.memset(ones_col[:], 1.0)
            ones_col_bf = cp.tile([1, 128], BF16, tag="ones_col_bf")
            nc.vector.memset(ones_col_bf[:], 1.0)
            ones12 = cp.tile([1, 12], F32, tag="ones12")
            nc.vector.memset(ones12[:], 1.0)
            iota_i = cp.tile([128, 512], I32, tag="iota_i")
            nc.gpsimd.iota(iota_i[:], pattern=[[1, 512]], base=0, channel_multiplier=0)
            iota_row = cp.tile([128, 512], F32, tag="iota_row")
            nc.vector.tensor_copy(iota_row[:], iota_i[:])
            pos_i = cp.tile([128, 4], I32, tag="pos_i")
            nc.gpsimd.iota(pos_i[:], pattern=[[128, 4]], base=0, channel_multiplier=1)
            pos_f = cp.tile([128, 4], F32, tag="pos_f")
            nc.vector.tensor_copy(pos_f[:], pos_i[:])
            pos_h = cp.tile([128, 4], FP16, tag="pos_h")
            nc.vector.tensor_copy(pos_h[:], pos_i[:])
            pad1e9 = cp.tile([1, 512], F32, tag="pad1e9")
            nc.vector.memset(pad1e9[:], 1.0e9)
            jlts = []
            for c in range(4):
                jl = cp.tile([128, 512], F32, tag=f"jlt{c}", name=f"jlt{c}")
                nc.vector.memset(jl[:], 1.0)
                nc.gpsimd.affine_select(
                    out=jl[:], in_=jl[:], compare_op=AL.is_gt,
                    fill=0.0, base=c * 128, pattern=[[-1, 512]],
                    channel_multiplier=1)
                jlts.append(jl)

            # ---------------- init: weights + batched q0 / V6 ----------
            xcls = smp.tile([4, C], F32, tag="xcls")
            for b in range(BL):
                nc.sync.dma_start(xcls[b:b + 1, :], x_d[b, 0:1, :])
            xclsT = smp.tile([128, 24], F32, tag="xclsT")
            xct_ps = ps_b.tile([128, 512], F32, tag="bigps")
            for j in range(NCH):
                nc.tensor.transpose(xct_ps[:, j * 4:(j + 1) * 4],
                                    xcls[0:4, j * 128:(j + 1) * 128],
                                    identf[0:4, 0:4])
            nc.scalar.copy(xclsT[:], xct_ps[:, 0:24])

            wqkv = []
            wproj = []
            q0_both = ps_s.tile([4, 2, 512], F32, tag="s2", bufs=2,
                                name="q0_both")
            q0_ps = q0_both[:, 0, :]
            q0_ps2 = q0_both[:, 1, 0:256]
            for j in range(NCH):
                wst = stp.tile([128, 3 * C], F32, tag="xstage0", name=f"wst{j}")
                nc.sync.dma_start(wst[:], wqkv_d[j * 128:(j + 1) * 128, :])
                w = wp.tile([128, 3 * C], BF16, tag=f"wqkv{j}", name=f"wqkv{j}")
                nc.vector.tensor_copy(w[:], wst[:])
                wqkv.append(w)
                nc.tensor.matmul(q0_ps, xclsT[:, j * 4:(j + 1) * 4],
                                 wst[:, 0:512],
                                 start=(j == 0), stop=(j == NCH - 1))
                nc.tensor.matmul(q0_ps2, xclsT[:, j * 4:(j + 1) * 4],
                                 wst[:, 512:C],
                                 start=(j == 0), stop=(j == NCH - 1))
            q0row = smp.tile([4, C], F32, tag="q0row")
            nc.scalar.copy(q0row[:, 0:512], q0_ps)
            nc.scalar.copy(q0row[:, 512:C], q0_ps2)
            for j in range(NCH):
                wst2 = stp.tile([128, C], F32, tag="xstage1", name=f"wst2{j}")
                nc.sync.dma_start(wst2[:], wproj_d[j * 128:(j + 1) * 128, :])
                w = wp.tile([128, C], BF16, tag=f"wproj{j}", name=f"wproj{j}")
                nc.vector.tensor_copy(w[:], wst2[:])
                wproj.append(w)
            # WkT (fp32, init-only): shares the xT tags so the space is
            # reclaimed once the per-sample xT tiles allocate.
            wkT = [php.tile([128, C], F32, tag=f"xT{bb}", name=f"wkT{bb}")
                   for bb in range(NCH)]
            for j in range(NCH):
                wst3 = stp.tile([128, C], F32, tag="xstage2", name=f"wst3{j}")
                nc.sync.dma_start(wst3[:], wqkv_d[j * 128:(j + 1) * 128, C:2 * C])
                wt_ps = ps_b.tile([128, 512], F32, tag="bigps")
                for bb in range(NCH):
                    sl = bb % 4
                    if bb == 4:
                        wt_ps = ps_b.tile([128, 512], F32, tag="bigps")
                    nc.tensor.transpose(
                        wt_ps[:, sl * 128:(sl + 1) * 128],
                        wst3[:, bb * 128:(bb + 1) * 128], identf[:])
                    nc.scalar.copy(wkT[bb][:, j * 128:(j + 1) * 128],
                                   wt_ps[:, sl * 128:(sl + 1) * 128])
            q0col = smp.tile([128, 24], F32, tag="q0col")
            q0c_ps = ps_b.tile([128, 512], F32, tag="bigps")
            for j in range(NCH):
                nc.tensor.transpose(q0c_ps[:, j * 4:(j + 1) * 4],
                                    q0row[0:4, j * 128:(j + 1) * 128],
                                    identf[0:4, 0:4])
            nc.scalar.copy(q0col[:], q0c_ps[:, 0:24])
            q0b_all = smp.tile([128, 6 * 48], F32, tag="q0b_all")
            nc.vector.memset(q0b_all[:], 0.0)
            for b in range(BL):
                for h in range(12):
                    j = h // 2
                    p0 = (h % 2) * 64
                    nc.vector.tensor_copy(
                        q0b_all[p0:p0 + 64, j * 48 + b * 12 + h:
                                j * 48 + b * 12 + h + 1],
                        q0col[p0:p0 + 64, j * 4 + b:j * 4 + b + 1])
            v6_all = []
            for a in range(NCH):
                v6_ps = ps_b.tile([128, 512], F32, tag="bigps")
                for bb in range(NCH):
                    nc.tensor.matmul(
                        v6_ps[:, 0:48],
                        wkT[bb][:, a * 128:(a + 1) * 128],
                        q0b_all[:, bb * 48:(bb + 1) * 48],
                        start=(bb == 0), stop=(bb == NCH - 1))
                v6a = wp.tile([128, 48], F32, tag=f"v6a{a}", name=f"v6a{a}")
                nc.scalar.copy(v6a[:], v6_ps[:, 0:48])
                v6_all.append(v6a)
            bias_row = smp.tile([1, C], F32, tag="bias_row")
            nc.sync.dma_start(bias_row[:], bproj_d[:])
            bias_bc = wp.tile([128, C], BF16, tag="bias_bc")
            for n0, n1 in ((0, 512), (512, 768)):
                bb_ps = ps_b.tile([128, 512], F32, tag="bigps")
                nc.tensor.matmul(bb_ps[:, :n1 - n0], ones_col[:],
                                 bias_row[:, n0:n1], start=True, stop=True)
                nc.scalar.copy(bias_bc[:, n0:n1], bb_ps[:, :n1 - n0])

            # ---------------- per-sample state ----------------
            state = {}

            def units_front(b):
                """Phases A-D + F for sample b, as emission units."""
                st = state[b] = {}
                units = []

                def u_load():
                    stg = [stp.tile([128, C], F32, tag=f"xstage{t}",
                                    name=f"stg{b}_{t}") for t in range(NTOK)]
                    for t in range(NTOK):
                        nc.sync.dma_start(stg[t][:],
                                          x_d[b, t * 128:(t + 1) * 128, :])
                    st["stg"] = stg
                    mask_i = smp.tile([1, 512], I32, tag="mask_i",
                                      name=f"mask_i{b}")
                    nc.sync.dma_start(mask_i[:], mrow_d[b:b + 1, :])
                    mask_f = smp.tile([1, 512], F32, tag="mask_f", bufs=2,
                                      name=f"mask_f{b}")
                    nc.vector.tensor_copy(mask_f[:], mask_i[:])
                    mcol_i = smp.tile([128, 4], I32, tag="mcol_i",
                                      name=f"mcol_i{b}")
                    nc.sync.dma_start(
                        mcol_i[:], mrow_d[b].rearrange("(c p) -> p c", p=128))
                    mcol = smp.tile([128, 4], F32, tag="mcol", bufs=2,
                                    name=f"mcol{b}")
                    nc.vector.tensor_copy(mcol[:], mcol_i[:])
                    st["mask_f"] = mask_f
                    st["mcol"] = mcol
                    st["xT"] = [php.tile([128, 512], F32, tag=f"xT{j}",
                                         name=f"xT{b}_{j}") for j in range(NCH)]
                    st["xTb"] = [php.tile([128, 512], BF16, tag=f"xTb{j}",
                                          name=f"xTb{b}_{j}") for j in range(NCH)]
                    st["qkT"] = [php.tile([128, 512], BF16, tag=f"qkT{m}",
                                          name=f"qkT{b}_{m}")
                                 for m in range(12)]
                    st["v_sb"] = [php.tile([128, 12 * 65], BF16, tag=f"v{t}",
                                           name=f"v{b}_{t}")
                                  for t in range(NTOK)]
                units.append(u_load)

                def u_transp(j):
                    def f():
                        xt_ps = ps_b.tile([128, 512], F32, tag="bigps")
                        for t in range(NTOK):
                            nc.tensor.transpose(
                                xt_ps[:, t * 128:(t + 1) * 128],
                                st["stg"][t][:, j * 128:(j + 1) * 128],
                                identf[:])
                        nc.scalar.copy(st["xT"][j][:], xt_ps[:])
                        nc.scalar.copy(st["xTb"][j][:], xt_ps[:])
                    return f
                units.extend(u_transp(j) for j in range(NCH))

                def u_qk(m):
                    def f():
                        qk_ps = ps_b.tile([128, 512], F32, tag="bigps")
                        for j in range(NCH):
                            nc.tensor.matmul(
                                qk_ps[:], wqkv[j][:, m * 128:(m + 1) * 128],
                                st["xTb"][j][:],
                                start=(j == 0), stop=(j == NCH - 1))
                        nc.scalar.copy(st["qkT"][m][:], qk_ps[:])
                    return f
                units.extend(u_qk(m) for m in range(12))

                def u_v(t):
                    def f():
                        for n in range(2):
                            v_ps = ps_b.tile([128, 512], F32, tag="bigps")
                            nf0 = 2 * C + n * 384
                            for j in range(NCH):
                                nc.tensor.matmul(
                                    v_ps[:, :384],
                                    st["xTb"][j][:, t * 128:(t + 1) * 128],
                                    wqkv[j][:, nf0:nf0 + 384],
                                    start=(j == 0), stop=(j == NCH - 1))
                            dst = st["v_sb"][t][:, n * 390:(n + 1) * 390]
                            dst = dst.rearrange("p (h d) -> p h d", h=6)[:, :, 0:64]
                            nc.vector.tensor_scalar(
                                dst,
                                v_ps[:, :384].rearrange("p (h d) -> p h d", h=6),
                                st["mcol"][:, t:t + 1], None,
                                op0=AL.mult, op1=AL.bypass)
                        mdst = st["v_sb"][t].rearrange(
                            "p (h d) -> p h d", h=12)[:, :, 64:65]
                        nc.vector.tensor_copy(
                            mdst, st["mcol"][:, t:t + 1].to_broadcast([128, 12, 1]))
                    return f
                units.extend(u_v(t) for t in range(NTOK))

                def u_scores():
                    scls_ps = ps_b.tile([128, 512], F32, tag="bigps")
                    for a in range(NCH):
                        nc.tensor.matmul(
                            scls_ps[0:12, :],
                            v6_all[a][:, b * 12:(b + 1) * 12],
                            st["xT"][a][:], start=(a == 0), stop=(a == NCH - 1))
                    mask_bc_ps = ps_b.tile([128, 512], F32, tag="bigps")
                    nc.tensor.matmul(mask_bc_ps[0:12, :], ones12[:],
                                     st["mask_f"][:], start=True, stop=True)
                    pexp = smp.tile([12, 512], F32, tag="pexp")
                    nc.scalar.activation(pexp[:], scls_ps[0:12, :], AF.Exp,
                                         scale=SCALE)
                    pm = smp.tile([12, 512], F32, tag="pm")
                    den12 = smp.tile([12, 1], F32, tag="den12")
                    nc.vector.scalar_tensor_tensor(
                        out=pm[:], in0=pexp[:], scalar=1.0,
                        in1=mask_bc_ps[0:12, :],
                        op0=AL.mult, op1=AL.mult, accum_out=den12[:])
                    rw = smp.tile([12, 1], F32, tag="rw")
                    nc.vector.reciprocal(rw[:], den12[:])
                    nc.vector.tensor_scalar_mul(rw[:], rw[:], 1.0 / 12.0)
                    sc_ps = ps_b.tile([128, 512], F32, tag="bigps")
                    nc.tensor.matmul(sc_ps[0:1, 0:NC1], rw[:], pm[:, 1:512],
                                     start=True, stop=True)
                    s_ext = smp.tile([1, 512], F32, tag="s_ext")
                    nc.scalar.copy(s_ext[0:1, 0:NC1], sc_ps[0:1, 0:NC1])
                    nc.vector.memset(s_ext[0:1, NC1:512], -1.0)
                    st["s_ext"] = s_ext
                units.append(u_scores)

                def u_rank1():
                    sb_ps = ps_b.tile([128, 512], F32, tag="bigps")
                    nc.tensor.matmul(sb_ps[:], ones_col[:], st["s_ext"][:],
                                     start=True, stop=True)
                    s_bc = smp.tile([128, 512], F32, tag="s_bc")
                    nc.scalar.copy(s_bc[:], sb_ps[:])
                    junk = smp.tile([128, 512], F32, tag="junk")
                    s_col = smp.tile([128, 4], F32, tag="s_col")
                    for c in range(4):
                        nc.vector.scalar_tensor_tensor(
                            out=junk[:], in0=iota_row[:],
                            scalar=pos_f[:, c:c + 1],
                            in1=s_bc[:], op0=AL.is_equal, op1=AL.mult,
                            accum_out=s_col[:, c:c + 1])
                    st["s_bc"] = s_bc
                    st["s_col"] = s_col
                    st["junk"] = junk
                units.append(u_rank1)

                def u_rank2():
                    s_bc, s_col, junk = st["s_bc"], st["s_col"], st["junk"]
                    # eq-stt shares the junk scratch; DVE ops serialize anyway
                    rank_col = smp.tile([128, 4], F32, tag="rank_col")
                    rank_eq = smp.tile([128, 4], F32, tag="rank_eq")
                    for c in range(4):
                        nc.vector.tensor_scalar(
                            junk[:], s_bc[:], s_col[:, c:c + 1], None,
                            op0=AL.is_gt, op1=AL.add,
                            accum_out=rank_col[:, c:c + 1])
                        nc.vector.scalar_tensor_tensor(
                            out=junk[:], in0=s_bc[:], scalar=s_col[:, c:c + 1],
                            in1=jlts[c][:], op0=AL.is_equal, op1=AL.mult,
                            accum_out=rank_eq[:, c:c + 1])
                    nc.vector.tensor_add(rank_col[:], rank_col[:], rank_eq[:])
                    idx_ps = ps_b.tile([128, 512], F32, tag="bigps")
                    for c in range(4):
                        eT = smp.tile([128, 512], FP16, tag="eT",
                                      name=f"eT{b}_{c}")
                        nc.vector.tensor_scalar(
                            eT[:], iota_row[:], rank_col[:, c:c + 1], None,
                            op0=AL.is_equal, op1=AL.bypass)
                        nc.tensor.matmul(idx_ps[0:1, :], pos_h[:, c:c + 1],
                                         eT[:], start=(c == 0), stop=(c == 3))
                    cnt = smp.tile([1, 1], F32, tag="cnt")
                    nc.vector.reduce_sum(cnt[:], st["mask_f"][0:1, 1:512],
                                         axis=mybir.AxisListType.X)
                    y = smp.tile([1, 1], F32, tag="y")
                    nc.vector.tensor_scalar_mul(y[:], cnt[:],
                                                float(np.float32(KEEP_RATE)))
                    sel = smp.tile([1, 512], I32, tag="sel")
                    nc.vector.tensor_scalar(
                        sel[:], iota_row[0:1, :], y[:, 0:1], None,
                        op0=AL.is_lt, op1=AL.bypass)
                    idx_sb = smp.tile([1, 512], F32, tag="idx_sb")
                    nc.scalar.copy(idx_sb[:], idx_ps[0:1, :])
                    idx_fin = smp.tile([1, 512], F32, tag="idx_fin")
                    nc.vector.select(idx_fin[:], sel[:], idx_sb[:], pad1e9[:])
                    nc.sync.dma_start(idx_d[b:b + 1, :],
                                      idx_fin[0:1, 0:BOUNDARY])
                units.append(u_rank2)
                return units

            def units_back(b):
                """Phases E (attention) + G (proj) for sample b."""
                st = state[b]
                units = []

                def mk_oallT():
                    st["oallT"] = [php.tile([128, 512], BF16, tag=f"oT{j}",
                                            name=f"oT{b}_{j}")
                                   for j in range(NCH)]
                units.append(mk_oallT)

                def u_pair_s(hp):
                    def f():
                        qkT = st["qkT"]
                        j = hp
                        p_chunks = []
                        for c in range(4):
                            s_ps = ps_s.tile([128, 2, 512], F32, tag="s2",
                                             bufs=2, name=f"s2_{b}_{hp}_{c}")
                            nc.tensor.matmul(
                                s_ps[:, 0, :],
                                qkT[6 + j][0:64, c * 128:(c + 1) * 128],
                                qkT[j][0:64, :], start=True, stop=True)
                            nc.tensor.matmul(
                                s_ps[:, 1, :],
                                qkT[6 + j][64:128, c * 128:(c + 1) * 128],
                                qkT[j][64:128, :], start=True, stop=True)
                            p_sb = stp.tile([128, 2, 512], BF16, tag="p_sb",
                                            bufs=8, name=f"p_{b}_{hp}_{c}")
                            nc.scalar.activation(p_sb[:], s_ps[:], AF.Exp,
                                                 scale=SCALE)
                            p_chunks.append(p_sb)
                        st[f"p{hp}"] = p_chunks
                    return f

                def u_pair_o(hp):
                    def f():
                        v_sb = st["v_sb"]
                        j = hp
                        p_chunks = st[f"p{hp}"]
                        for hh in range(2):
                            h = 2 * hp + hh
                            p0 = hh * 64
                            o_ps = ps_o.tile([65, 512], F32, tag="ops")
                            for c in range(4):
                                nc.tensor.matmul(
                                    o_ps[:], v_sb[c][:, h * 65:h * 65 + 65],
                                    p_chunks[c][:, hh, :],
                                    start=(c == 0), stop=(c == 3))
                            den_row = stp.tile([1, 512], F32, tag="den_row",
                                               bufs=2)
                            nc.vector.tensor_copy(den_row[:], o_ps[64:65, :])
                            rec_row = stp.tile([1, 512], F32, tag="rec_row",
                                               bufs=2)
                            nc.vector.reciprocal_approx_fast(out=rec_row[:],
                                                             in_=den_row[:])
                            rec_bf = stp.tile([1, 512], BF16, tag="rec_bf",
                                              bufs=2)
                            nc.vector.tensor_copy(rec_bf[:], rec_row[:])
                            rb_ps = ps_o.tile([65, 512], F32, tag="ops",
                                              name=f"rb_{b}_{h}")
                            nc.tensor.matmul(rb_ps[0:64, :],
                                             ones_col_bf[:, 0:64],
                                             rec_bf[:], start=True, stop=True)
                            rb_sb = stp.tile([64, 512], F32, tag="rb_sb",
                                             bufs=2)
                            nc.scalar.copy(rb_sb[:], rb_ps[0:64, :])
                            nc.vector.tensor_tensor(
                                out=st["oallT"][j][p0:p0 + 64, :],
                                in0=o_ps[0:64, :], in1=rb_sb[:], op=AL.mult)
                    return f

                # skewed pipeline: S(hp+1) emitted before O(hp)
                units.append(u_pair_s(0))
                for hp in range(1, 6):
                    units.append(u_pair_s(hp))
                    units.append(u_pair_o(hp - 1))
                units.append(u_pair_o(5))

                def u_proj(t):
                    def f():
                        pr_sb = pop.tile([128, C], F32, tag="proj")
                        for n in range(2):
                            pr_ps = ps_b.tile([128, 512], F32, tag="bigps")
                            n0 = n * 384
                            for j in range(NCH):
                                nc.tensor.matmul(
                                    pr_ps[:, :384],
                                    st["oallT"][j][:, t * 128:(t + 1) * 128],
                                    wproj[j][:, n0:n0 + 384],
                                    start=(j == 0), stop=(j == NCH - 1))
                            nc.vector.tensor_tensor(
                                out=pr_sb[:, n0:n0 + 384], in0=pr_ps[:, :384],
                                in1=bias_bc[:, n0:n0 + 384], op=AL.add)
                        nc.sync.dma_start(
                            out_d[b, t * 128:(t + 1) * 128, :], pr_sb[:])
                    return f
                units.extend(u_proj(t) for t in range(NTOK))
                return units

            # serial phase emission; pipelining is inside phase E
            for b in range(BL):
                for u in units_front(b):
                    u()
                for u in units_back(b):
                    u()

    nc.compile()
    return nc


def _get_nc():
    if "nc" not in _CACHE:
        _CACHE["nc"] = _build_nc()
    return _CACHE["nc"]


def _make_in_maps(inputs):
    x = np.ascontiguousarray(np.asarray(inputs["x"], dtype=np.float32))
    mask_row = np.ascontiguousarray(
        np.asarray(inputs["attn_mask"], dtype=np.int32)[:, 0, :])
    w_qkv = np.ascontiguousarray(np.asarray(inputs["w_qkv"], dtype=np.float32))
    w_proj = np.ascontiguousarray(np.asarray(inputs["w_proj"], dtype=np.float32))
    b_proj = np.ascontiguousarray(
        np.asarray(inputs["b_proj"], dtype=np.float32).reshape(1, C))
    in_maps = []
    for i in range(NCORES):
        sl = slice(i * BL, (i + 1) * BL)
        in_maps.append({
            "x": x[sl],
            "mask_row": mask_row[sl],
            "w_qkv": w_qkv,
            "w_proj": w_proj,
            "b_proj": b_proj,
        })
    return in_maps


def run_on_device(inputs, trace=False, tmpdir=None):
    """Build + run on the 8 NeuronCores; returns (out, idx, exec_time_ns)."""
    from concourse.bass_utils import run_bass_kernel_spmd

    nc = _get_nc()
    in_maps = _make_in_maps(inputs)
    res = run_bass_kernel_spmd(
        nc, in_maps, core_ids=list(range(NCORES)), trace=trace, tmpdir=tmpdir)
    out = np.concatenate([res.results[i]["out_x"] for i in range(NCORES)], axis=0)
    idx = np.concatenate([res.results[i]["out_idx"] for i in range(NCORES)], axis=0)
    return out, idx, res.exec_time_ns


def kernel(**inputs):
    out, idx, _ = run_on_device(inputs, trace=False)
    index = np.ascontiguousarray(
        np.broadcast_to(idx[:, :, None], (B, BOUNDARY, C)))
    return out, index, idx, BOUNDARY


# revision 21
# speedup vs baseline: 1.1186x; 1.0779x over previous
"""Trainium2 Bass kernel for nn_Attention_66108136620268.

Full multi-head attention (B=32, N=512, C=768, H=12) with CLS-attention
top-k token pruning, data-parallel over batch across 8 NeuronCores.

Main attention/proj path runs in bf16 (out tolerance is loose); the
CLS-score path that determines the top-k ordering is computed in exact
fp32 via a fused form:  l[h,k] = sum_a V6[a,h] * xT[a,k]  with
V6 = Wk @ blockdiag(q0),  q0 = Wq^T @ x[:,0,:].  q0/V6 depend only on
the CLS rows of x, so they are batched over all samples at kernel init.

Top-k is computed as an exact rank sort: rank[i] = #{s_j > s_i} +
#{j<i: s_j == s_i} via fused compare+accumulate DVE ops, then inverted
with an equality-match matmul; positions >= ceil(0.7*valid) padded 1e9.

The per-sample pipeline is software-pipelined at emission level: the
attention/normalize phase of sample b-1 is interleaved with the
projection phases of sample b so the TensorEngine sees dense work.

kernel(**inputs) takes the FULL inputs, shards over 8 cores, returns
(out, index, idx, BOUNDARY) matching the reference.
"""

import numpy as np

B, N, C, H = 32, 512, 768, 12
HD = C // H                      # 64
SCALE = float(HD ** -0.5)        # 0.125
KEEP_RATE = 0.7
BOUNDARY = 358
NCORES = 8
BL = B // NCORES                 # 4 samples per core
NC1 = N - 1                      # 511

_CACHE = {}


def _build_nc():
    import concourse.bacc as bacc
    import concourse.bass as bass
    import concourse.mybir as mybir
    import concourse.tile as tile
    from concourse.masks import make_identity

    F32 = mybir.dt.float32
    BF16 = mybir.dt.bfloat16
    FP16 = mybir.dt.float16
    I32 = mybir.dt.int32
    AL = mybir.AluOpType
    AF = mybir.ActivationFunctionType
    PSUM = bass.MemorySpace.PSUM

    nc = bacc.Bacc()

    x_d = nc.declare_dram_parameter("x", [BL, N, C], F32, isOutput=False)
    mrow_d = nc.declare_dram_parameter("mask_row", [BL, N], I32, isOutput=False)
    wqkv_d = nc.declare_dram_parameter("w_qkv", [C, 3 * C], F32, isOutput=False)
    wproj_d = nc.declare_dram_parameter("w_proj", [C, C], F32, isOutput=False)
    bproj_d = nc.declare_dram_parameter("b_proj", [1, C], F32, isOutput=False)
    out_d = nc.declare_dram_parameter("out_x", [BL, N, C], F32, isOutput=True)
    idx_d = nc.declare_dram_parameter("out_idx", [BL, BOUNDARY], F32, isOutput=True)

    NCH = C // 128               # 6 c-chunks
    NTOK = N // 128              # 4 token chunks

    with tile.TileContext(nc) as tc:
        with (
            tc.tile_pool(name="const", bufs=1) as cp,
            tc.tile_pool(name="wp", bufs=1) as wp,
            tc.tile_pool(name="st", bufs=1) as stp,
            tc.tile_pool(name="ph", bufs=1) as php,
            tc.tile_pool(name="sm", bufs=1) as smp,
            tc.tile_pool(name="po", bufs=2) as pop,
            tc.tile_pool(name="ps_s", bufs=1, space=PSUM) as ps_s,
            tc.tile_pool(name="ps_o", bufs=2, space=PSUM) as ps_o,
            tc.tile_pool(name="ps_b", bufs=2, space=PSUM) as ps_b,
        ):
            # ---------------- constants ----------------
            identf = cp.tile([128, 128], F32, tag="identf")
            make_identity(nc, identf[:])
            ones_col = cp.tile([1, 128], F32, tag="ones_col")
            nc.vector.memset(ones_col[:], 1.0)
            ones_col_bf = cp.tile([1, 128], BF16, tag="ones_col_bf")
            nc.vector.memset(ones_col_bf[:], 1.0)
            ones12 = cp.tile([1, 12], F32, tag="ones12")
            nc.vector.memset(ones12[:], 1.0)
            iota_i = cp.tile([128, 512], I32, tag="iota_i")
            nc.gpsimd.iota(iota_i[:], pattern=[[1, 512]], base=0, channel_multiplier=0)
            iota_row = cp.tile([128, 512], F32, tag="iota_row")
            nc.vector.tensor_copy(iota_row[:], iota_i[:])
            pos_i = cp.tile([128, 4], I32, tag="pos_i")
            nc.gpsimd.iota(pos_i[:], pattern=[[128, 4]], base=0, channel_multiplier=1)
            pos_f = cp.tile([128, 4], F32, tag="pos_f")
            nc.vector.tensor_copy(pos_f[:], pos_i[:])
            pos_h = cp.tile([128, 4], FP16, tag="pos_h")
            nc.vector.tensor_copy(pos_h[:], pos_i[:])
            pad1e9 = cp.tile([1, 512], F32, tag="pad1e9")
            nc.vector.memset(pad1e9[:], 1.0e9)
            jlts = []
            for c in range(4):
                jl = cp.tile([128, 512], F32, tag=f"jlt{c}", name=f"jlt{c}")
                nc.vector.memset(jl[:], 1.0)
                nc.gpsimd.affine_select(
                    out=jl[:], in_=jl[:], compare_op=AL.is_gt,
                    fill=0.0, base=c * 128, pattern=[[-1, 512]],
                    channel_multiplier=1)
                jlts.append(jl)

            # ---------------- init: weights + batched q0 / V6 ----------
            xcls = smp.tile([4, C], F32, tag="xcls")
            for b in range(BL):
                nc.sync.dma_start(xcls[b:b + 1, :], x_d[b, 0:1, :])
            xclsT = smp.tile([128, 24], F32, tag="xclsT")
            xct_ps = ps_b.tile([128, 512], F32, tag="bigps")
            for j in range(NCH):
                nc.tensor.transpose(xct_ps[:, j * 4:(j + 1) * 4],
                                    xcls[0:4, j * 128:(j + 1) * 128],
                                    identf[0:4, 0:4])
            nc.scalar.copy(xclsT[:], xct_ps[:, 0:24])

            wqkv = []
            wproj = []
            q0_both = ps_s.tile([4, 2, 512], F32, tag="s2", bufs=2,
                                name="q0_both")
            q0_ps = q0_both[:, 0, :]
            q0_ps2 = q0_both[:, 1, 0:256]
            for j in range(NCH):
                wst = stp.tile([128, 3 * C], F32, tag="xstage0", name=f"wst{j}")
                nc.sync.dma_start(wst[:], wqkv_d[j * 128:(j + 1) * 128, :])
                w = wp.tile([128, 3 * C], BF16, tag=f"wqkv{j}", name=f"wqkv{j}")
                nc.vector.tensor_copy(w[:], wst[:])
                wqkv.append(w)
                nc.tensor.matmul(q0_ps, xclsT[:, j * 4:(j + 1) * 4],
                                 wst[:, 0:512],
                                 start=(j == 0), stop=(j == NCH - 1))
                nc.tensor.matmul(q0_ps2, xclsT[:, j * 4:(j + 1) * 4],
                                 wst[:, 512:C],
                                 start=(j == 0), stop=(j == NCH - 1))
            q0row = smp.tile([4, C], F32, tag="q0row")
            nc.scalar.copy(q0row[:, 0:512], q0_ps)
            nc.scalar.copy(q0row[:, 512:C], q0_ps2)
            for j in range(NCH):
                wst2 = stp.tile([128, C], F32, tag="xstage1", name=f"wst2{j}")
                nc.sync.dma_start(wst2[:], wproj_d[j * 128:(j + 1) * 128, :])
                w = wp.tile([128, C], BF16, tag=f"wproj{j}", name=f"wproj{j}")
                nc.vector.tensor_copy(w[:], wst2[:])
                wproj.append(w)
            # WkT (fp32, init-only): shares the xT tags so the space is
            # reclaimed once the per-sample xT tiles allocate.
            wkT = [php.tile([128, C], F32, tag=f"xT{bb}", name=f"wkT{bb}")
                   for bb in range(NCH)]
            for j in range(NCH):
                wst3 = stp.tile([128, C], F32, tag="xstage2", name=f"wst3{j}")
                nc.sync.dma_start(wst3[:], wqkv_d[j * 128:(j + 1) * 128, C:2 * C])
                wt_ps = ps_b.tile([128, 512], F32, tag="bigps")
                for bb in range(NCH):
                    sl = bb % 4
                    if bb == 4:
                        wt_ps = ps_b.tile([128, 512], F32, tag="bigps")
                    nc.tensor.transpose(
                        wt_ps[:, sl * 128:(sl + 1) * 128],
                        wst3[:, bb * 128:(bb + 1) * 128], identf[:])
                    nc.scalar.copy(wkT[bb][:, j * 128:(j + 1) * 128],
                                   wt_ps[:, sl * 128:(sl + 1) * 128])
            q0col = smp.tile([128, 24], F32, tag="q0col")
            q0c_ps = ps_b.tile([128, 512], F32, tag="bigps")
            for j in range(NCH):
                nc.tensor.transpose(q0c_ps[:, j * 4:(j + 1) * 4],
                                    q0row[0:4, j * 128:(j + 1) * 128],
                                    identf[0:4, 0:4])
            nc.scalar.copy(q0col[:], q0c_ps[:, 0:24])
            q0b_all = smp.tile([128, 6 * 48], F32, tag="q0b_all")
            nc.vector.memset(q0b_all[:], 0.0)
            for b in range(BL):
                for h in range(12):
                    j = h // 2
                    p0 = (h % 2) * 64
                    nc.vector.tensor_copy(
                        q0b_all[p0:p0 + 64, j * 48 + b * 12 + h:
                                j * 48 + b * 12 + h + 1],
                        q0col[p0:p0 + 64, j * 4 + b:j * 4 + b + 1])
            v6_all = []
            for a in range(NCH):
                v6_ps = ps_b.tile([128, 512], F32, tag="bigps")
                for bb in range(NCH):
                    nc.tensor.matmul(
                        v6_ps[:, 0:48],
                        wkT[bb][:, a * 128:(a + 1) * 128],
                        q0b_all[:, bb * 48:(bb + 1) * 48],
                        start=(bb == 0), stop=(bb == NCH - 1))
                v6a = wp.tile([128, 48], F32, tag=f"v6a{a}", name=f"v6a{a}")
                nc.scalar.copy(v6a[:], v6_ps[:, 0:48])
                v6_all.append(v6a)
            bias_row = smp.tile([1, C], F32, tag="bias_row")
            nc.sync.dma_start(bias_row[:], bproj_d[:])
            bias_bc = wp.tile([128, C], BF16, tag="bias_bc")
            for n0, n1 in ((0, 512), (512, 768)):
                bb_ps = ps_b.tile([128, 512], F32, tag="bigps")
                nc.tensor.matmul(bb_ps[:, :n1 - n0], ones_col[:],
                                 bias_row[:, n0:n1], start=True, stop=True)
                nc.scalar.copy(bias_bc[:, n0:n1], bb_ps[:, :n1 - n0])

            # ---------------- per-sample state ----------------
            state = {}

            def units_front(b):
                """Phases A-D + F for sample b, as emission units."""
                st = state[b] = {}
                units = []

                def u_load():
                    stg = [stp.tile([128, C], F32, tag=f"xstage{t}",
                                    name=f"stg{b}_{t}") for t in range(NTOK)]
                    for t in range(NTOK):
                        nc.sync.dma_start(stg[t][:],
                                          x_d[b, t * 128:(t + 1) * 128, :])
                    st["stg"] = stg
                    mask_i = smp.tile([1, 512], I32, tag="mask_i",
                                      name=f"mask_i{b}")
                    nc.sync.dma_start(mask_i[:], mrow_d[b:b + 1, :])
                    mask_f = smp.tile([1, 512], F32, tag="mask_f", bufs=2,
                                      name=f"mask_f{b}")
                    nc.vector.tensor_copy(mask_f[:], mask_i[:])
                    mcol_i = smp.tile([128, 4], I32, tag="mcol_i",
                                      name=f"mcol_i{b}")
                    nc.sync.dma_start(
                        mcol_i[:], mrow_d[b].rearrange("(c p) -> p c", p=128))
                    mcol = smp.tile([128, 4], F32, tag="mcol", bufs=2,
                                    name=f"mcol{b}")
                    nc.vector.tensor_copy(mcol[:], mcol_i[:])
                    st["mask_f"] = mask_f
                    st["mcol"] = mcol
                    st["xT"] = [php.tile([128, 512], F32, tag=f"xT{j}",
                                         name=f"xT{b}_{j}") for j in range(NCH)]
                    st["xTb"] = [php.tile([128, 512], BF16, tag=f"xTb{j}",
                                          name=f"xTb{b}_{j}") for j in range(NCH)]
                    st["qkT"] = [php.tile([128, 512], BF16, tag=f"qkT{m}",
                                          name=f"qkT{b}_{m}")
                                 for m in range(12)]
                    st["v_sb"] = [php.tile([128, 12 * 65], BF16, tag=f"v{t}",
                                           name=f"v{b}_{t}")
                                  for t in range(NTOK)]
                units.append(u_load)

                def u_transp(j):
                    def f():
                        xt_ps = ps_b.tile([128, 512], F32, tag="bigps")
                        for t in range(NTOK):
                            nc.tensor.transpose(
                                xt_ps[:, t * 128:(t + 1) * 128],
                                st["stg"][t][:, j * 128:(j + 1) * 128],
                                identf[:])
                        nc.scalar.copy(st["xT"][j][:], xt_ps[:])
                        nc.scalar.copy(st["xTb"][j][:], xt_ps[:])
                    return f
                units.extend(u_transp(j) for j in range(NCH))

                def u_qk(m):
                    def f():
                        qk_ps = ps_b.tile([128, 512], F32, tag="bigps")
                        for j in range(NCH):
                            nc.tensor.matmul(
                                qk_ps[:], wqkv[j][:, m * 128:(m + 1) * 128],
                                st["xTb"][j][:],
                                start=(j == 0), stop=(j == NCH - 1))
                        nc.scalar.copy(st["qkT"][m][:], qk_ps[:])
                    return f
                units.extend(u_qk(m) for m in range(12))

                def u_v(t):
                    def f():
                        for n in range(2):
                            v_ps = ps_b.tile([128, 512], F32, tag="bigps")
                            nf0 = 2 * C + n * 384
                            for j in range(NCH):
                                nc.tensor.matmul(
                                    v_ps[:, :384],
                                    st["xTb"][j][:, t * 128:(t + 1) * 128],
                                    wqkv[j][:, nf0:nf0 + 384],
                                    start=(j == 0), stop=(j == NCH - 1))
                            dst = st["v_sb"][t][:, n * 390:(n + 1) * 390]
                            dst = dst.rearrange("p (h d) -> p h d", h=6)[:, :, 0:64]
                            nc.vector.tensor_scalar(
                                dst,
                                v_ps[:, :384].rearrange("p (h d) -> p h d", h=6),
                                st["mcol"][:, t:t + 1], None,
                                op0=AL.mult, op1=AL.bypass)
                        mdst = st["v_sb"][t].rearrange(
                            "p (h d) -> p h d", h=12)[:, :, 64:65]
                        nc.vector.tensor_copy(
                            mdst, st["mcol"][:, t:t + 1].to_broadcast([128, 12, 1]))
                    return f
                units.extend(u_v(t) for t in range(NTOK))

                def u_scores():
                    scls_ps = ps_b.tile([128, 512], F32, tag="bigps")
                    for a in range(NCH):
                        nc.tensor.matmul(
                            scls_ps[0:12, :],
                            v6_all[a][:, b * 12:(b + 1) * 12],
                            st["xT"][a][:], start=(a == 0), stop=(a == NCH - 1))
                    mask_bc_ps = ps_b.tile([128, 512], F32, tag="bigps")
                    nc.tensor.matmul(mask_bc_ps[0:12, :], ones12[:],
                                     st["mask_f"][:], start=True, stop=True)
                    pexp = smp.tile([12, 512], F32, tag="pexp")
                    nc.scalar.activation(pexp[:], scls_ps[0:12, :], AF.Exp,
                                         scale=SCALE)
                    pm = smp.tile([12, 512], F32, tag="pm")
                    den12 = smp.tile([12, 1], F32, tag="den12")
                    nc.vector.scalar_tensor_tensor(
                        out=pm[:], in0=pexp[:], scalar=1.0,
                        in1=mask_bc_ps[0:12, :],
                        op0=AL.mult, op1=AL.mult, accum_out=den12[:])
                    rw = smp.tile([12, 1], F32, tag="rw")
                    nc.vector.reciprocal(rw[:], den12[:])
                    nc.vector.tensor_scalar_mul(rw[:], rw[:], 1.0 / 12.0)
                    sc_ps = ps_b.tile([128, 512], F32, tag="bigps")
                    nc.tensor.matmul(sc_ps[0:1, 0:NC1], rw[:], pm[:, 1:512],
                                     start=True, stop=True)
                    s_ext = smp.tile([1, 512], F32, tag="s_ext")
                    nc.scalar.copy(s_ext[0:1, 0:NC1], sc_ps[0:1, 0:NC1])
                    nc.vector.memset(s_ext[0:1, NC1:512], -1.0)
                    st["s_ext"] = s_ext
                units.append(u_scores)

                def u_rank1():
                    sb_ps = ps_b.tile([128, 512], F32, tag="bigps")
                    nc.tensor.matmul(sb_ps[:], ones_col[:], st["s_ext"][:],
                                     start=True, stop=True)
                    s_bc = smp.tile([128, 512], F32, tag="s_bc")
                    nc.scalar.copy(s_bc[:], sb_ps[:])
                    junk = smp.tile([128, 512], F32, tag="junk")
                    s_col = smp.tile([128, 4], F32, tag="s_col")
                    for c in range(4):
                        nc.vector.scalar_tensor_tensor(
                            out=junk[:], in0=iota_row[:],
                            scalar=pos_f[:, c:c + 1],
                            in1=s_bc[:], op0=AL.is_equal, op1=AL.mult,
                            accum_out=s_col[:, c:c + 1])
                    st["s_bc"] = s_bc
                    st["s_col"] = s_col
                    st["junk"] = junk
                units.append(u_rank1)

                def u_rank2():
                    s_bc, s_col, junk = st["s_bc"], st["s_col"], st["junk"]
                    # eq-stt shares the junk scratch; DVE ops serialize anyway
                    rank_col = smp.tile([128, 4], F32, tag="rank_col")
                    rank_eq = smp.tile([128, 4], F32, tag="rank_eq")
                    for c in range(4):
                        nc.vector.tensor_scalar(
                            junk[:], s_bc[:], s_col[:, c:c + 1], None,
                            op0=AL.is_gt, op1=AL.add,
                            accum_out=rank_col[:, c:c + 1])
                        nc.vector.scalar_tensor_tensor(
                            out=junk[:], in0=s_bc[:], scalar=s_col[:, c:c + 1],
                            in1=jlts[c][:], op0=AL.is_equal, op1=AL.mult,
                            accum_out=rank_eq[:, c:c + 1])
                    nc.vector.tensor_add(rank_col[:], rank_col[:], rank_eq[:])
                    idx_ps = ps_b.tile([128, 512], F32, tag="bigps")
                    for c in range(4):
                        eT = smp.tile([128, 512], FP16, tag="eT",
                                      name=f"eT{b}_{c}")
                        nc.vector.tensor_scalar(
                            eT[:], iota_row[:], rank_col[:, c:c + 1], None,
                            op0=AL.is_equal, op1=AL.bypass)
                        nc.tensor.matmul(idx_ps[0:1, :], pos_h[:, c:c + 1],
                                         eT[:], start=(c == 0), stop=(c == 3))
                    cnt = smp.tile([1, 1], F32, tag="cnt")
                    nc.vector.reduce_sum(cnt[:], st["mask_f"][0:1, 1:512],
                                         axis=mybir.AxisListType.X)
                    y = smp.tile([1, 1], F32, tag="y")
                    nc.vector.tensor_scalar_mul(y[:], cnt[:],
                                                float(np.float32(KEEP_RATE)))
                    sel = smp.tile([1, 512], I32, tag="sel")
                    nc.vector.tensor_scalar(
                        sel[:], iota_row[0:1, :], y[:, 0:1], None,
                        op0=AL.is_lt, op1=AL.bypass)
                    idx_sb = smp.tile([1, 512], F32, tag="idx_sb")
                    nc.scalar.copy(idx_sb[:], idx_ps[0:1, :])
                    idx_fin = smp.tile([1, 512], F32, tag="idx_fin")
                    nc.vector.select(idx_fin[:], sel[:], idx_sb[:], pad1e9[:])
                    nc.sync.dma_start(idx_d[b:b + 1, :],
                                      idx_fin[0:1, 0:BOUNDARY])
                units.append(u_rank2)
                return units

            def units_back(b):
                """Phases E (attention) + G (proj) for sample b."""
                st = state[b]
                units = []

                def mk_oallT():
                    st["oallT"] = [php.tile([128, 512], BF16, tag=f"oT{j}",
                                            name=f"oT{b}_{j}")
                                   for j in range(NCH)]
                units.append(mk_oallT)

                def u_head(h):
                    def f():
                        qkT = st["qkT"]
                        v_sb = st["v_sb"]
                        j = h // 2
                        p0 = (h % 2) * 64
                        p_half = []
                        for half in range(2):
                            s_ps = ps_s.tile([128, 2, 512], F32, tag="s2",
                                             bufs=2, name=f"s2_{b}_{h}_{half}")
                            for cc in range(2):
                                c = half * 2 + cc
                                nc.tensor.matmul(
                                    s_ps[:, cc, :],
                                    qkT[6 + j][p0:p0 + 64, c * 128:(c + 1) * 128],
                                    qkT[j][p0:p0 + 64, :],
                                    start=True, stop=True)
                            p_sb = stp.tile([128, 2, 512], BF16, tag="p_sb",
                                            bufs=4, name=f"p_{b}_{h}_{half}")
                            nc.scalar.activation(p_sb[:], s_ps[:], AF.Exp,
                                                 scale=SCALE)
                            p_half.append(p_sb)
                        o_ps = ps_o.tile([65, 512], F32, tag="ops")
                        for c in range(4):
                            nc.tensor.matmul(
                                o_ps[:], v_sb[c][:, h * 65:h * 65 + 65],
                                p_half[c // 2][:, c % 2, :],
                                start=(c == 0), stop=(c == 3))
                        den_row = stp.tile([1, 512], F32, tag="den_row", bufs=2)
                        nc.scalar.copy(den_row[:], o_ps[64:65, :])
                        rec_row = stp.tile([1, 512], F32, tag="rec_row", bufs=2)
                        nc.vector.reciprocal_approx_fast(out=rec_row[:],
                                                         in_=den_row[:])
                        rec_bf = stp.tile([1, 512], BF16, tag="rec_bf", bufs=2)
                        nc.vector.tensor_copy(rec_bf[:], rec_row[:])
                        rb_ps = ps_b.tile([128, 512], F32, tag="bigps")
                        nc.tensor.matmul(rb_ps[0:64, :], ones_col_bf[:, 0:64],
                                         rec_bf[:], start=True, stop=True)
                        rb_sb = stp.tile([64, 512], F32, tag="rb_sb", bufs=2)
                        nc.scalar.copy(rb_sb[:], rb_ps[0:64, :])
                        nc.vector.tensor_tensor(
                            out=st["oallT"][j][p0:p0 + 64, :],
                            in0=o_ps[0:64, :], in1=rb_sb[:], op=AL.mult)
                    return f
                units.extend(u_head(h) for h in range(12))

                def u_proj(t):
                    def f():
                        pr_sb = pop.tile([128, C], F32, tag="proj")
                        for n in range(2):
                            pr_ps = ps_b.tile([128, 512], F32, tag="bigps")
                            n0 = n * 384
                            for j in range(NCH):
                                nc.tensor.matmul(
                                    pr_ps[:, :384],
                                    st["oallT"][j][:, t * 128:(t + 1) * 128],
                                    wproj[j][:, n0:n0 + 384],
                                    start=(j == 0), stop=(j == NCH - 1))
                            nc.vector.tensor_tensor(
                                out=pr_sb[:, n0:n0 + 384], in0=pr_ps[:, :384],
                                in1=bias_bc[:, n0:n0 + 384], op=AL.add)
                        nc.sync.dma_start(
                            out_d[b, t * 128:(t + 1) * 128, :], pr_sb[:])
                    return f
                units.extend(u_proj(t) for t in range(NTOK))
                return units

            # serial phase emission; pipelining is inside phase E
            for b in range(BL):
                for u in units_front(b):
                    u()
                for u in units_back(b):
                    u()

    nc.compile()
    return nc


def _get_nc():
    if "nc" not in _CACHE:
        _CACHE["nc"] = _build_nc()
    return _CACHE["nc"]


def _make_in_maps(inputs):
    x = np.ascontiguousarray(np.asarray(inputs["x"], dtype=np.float32))
    mask_row = np.ascontiguousarray(
        np.asarray(inputs["attn_mask"], dtype=np.int32)[:, 0, :])
    w_qkv = np.ascontiguousarray(np.asarray(inputs["w_qkv"], dtype=np.float32))
    w_proj = np.ascontiguousarray(np.asarray(inputs["w_proj"], dtype=np.float32))
    b_proj = np.ascontiguousarray(
        np.asarray(inputs["b_proj"], dtype=np.float32).reshape(1, C))
    in_maps = []
    for i in range(NCORES):
        sl = slice(i * BL, (i + 1) * BL)
        in_maps.append({
            "x": x[sl],
            "mask_row": mask_row[sl],
            "w_qkv": w_qkv,
            "w_proj": w_proj,
            "b_proj": b_proj,
        })
    return in_maps


def run_on_device(inputs, trace=False, tmpdir=None):
    """Build + run on the 8 NeuronCores; returns (out, idx, exec_time_ns)."""
    from concourse.bass_utils import run_bass_kernel_spmd

    nc = _get_nc()
    in_maps = _make_in_maps(inputs)
    res = run_bass_kernel_spmd(
        nc, in_maps, core_ids=list(range(NCORES)), trace=trace, tmpdir=tmpdir)
    out = np.concatenate([res.results[i]["out_x"] for i in range(NCORES)], axis=0)
    idx = np.concatenate([res.results[i]["out_idx"] for i in range(NCORES)], axis=0)
    return out, idx, res.exec_time_ns


def kernel(**inputs):
    out, idx, _ = run_on_device(inputs, trace=False)
    index = np.ascontiguousarray(
        np.broadcast_to(idx[:, :, None], (B, BOUNDARY, C)))
    return out, index, idx, BOUNDARY
